# revision 29
# baseline (speedup 1.0000x reference)
"""BFP-quantized linear (nn_BFPLinear) on 8 Trainium2 NeuronCores.

Math (must match reference exactly):
    xq = bfp_quant8_g64(x); wq = bfp_quant8_g64(weight)
    out = xq @ wq.T + 2*bias

Sharding (2 row-groups x 4 col-groups grid, core c = 4r+k), NO collectives:
  - core (r, k) owns x rows [2048r, 2048(r+1)) and w rows [1024k, 1024(k+1)).
  - Each core quantizes its own x/w slices (w slices are quantized
    redundantly by the 2 cores of each column group — cheaper than any
    cross-core exchange, and leaves the cores fully independent).
  - Core output: out[2048r:2048(r+1), 1024k:1024(k+1)] in natural row
    order; the host just places the 8 blocks.

Quantization per group of 64 along `in` (bit-exact vs reference):
    gmax = max|x|; e = floor(log2(gmax)) via exponent-bit masking;
    step = 2^(e-7), inv = 2^(7-e) (exact bit arithmetic);
    m = clip(round_half_even(x*inv), -128, 127) via pre-clip to
    [-128.49, 127.49] + the fp32 magic-number round (+-1.5*2^23);
    xq = m * step, exact in bf16 (|m| <= 128, step = power of 2).

Layout for the matmul: both operands need the contraction dim (`in`) on
partitions, so xq/wq tiles [128 rows, 4096] are transposed with the DMA
xbar into [128 in-part, kt, row] tiles (SBUF->SBUF, no HBM round trip).
"""
import sys

sys.path.insert(0, "/opt/trn_rl_repo")

import numpy as np

import concourse.bass as bass
import concourse.tile as tile
from concourse import mybir, bacc
from concourse.bass_utils import run_bass_kernel_spmd

# problem shape (hardcoded; kernel.py must be self-contained)
N = 4096
IN = 4096
OUT = 4096
NCORES = 8
RGRP = 2                 # row groups (x sharded 2-way)
CGRP = 4                 # col groups (w sharded 4-way)
XROWS = N // RGRP        # 2048 x rows per core
WROWS = OUT // CGRP      # 1024 w rows per core
P = 128
J = 64                   # bfp group size
KT = IN // P             # 32 contraction k-tiles
XT = XROWS // P          # 16 x row-tiles per core
WT = WROWS // P          # 8 w row-tiles per core
HF = 2048                # quantize sub-tile width (along `in`)
NH = IN // HF            # 2 halves per row-tile
KH = HF // P             # 16 k-tiles per half

# fp16 bit-field constants (inputs are shipped as fp16; exponent math is
# done on int16 views, producing bf16 inv/step: bits catch 2^(7-e)/2^(e-7))
MASK16 = 0x7C00          # fp16 exponent field
MINN16 = 0x0400          # smallest normal fp16 (e = -14 floor)
INVC16 = 0x4A80          # bf16 bits: (149 - E16) << 7 == 2^(7-e)
STEPC16 = 0x3480         # bf16 bits: (E16 + 105) << 7 == 2^(e-7)
MAGIC16 = 1536.0         # 1.5 * 2^10: fp16 ulp-1 octave [1024, 2048)

# transpose path: True = SBUF->SBUF xbar (no HBM round trip);
# False = stage quantized tiles in DRAM and read back transposed.
XBAR_SBUF = True

import os
OHALF = int(os.environ.get("K_OHALF", "512"))
XHEAD = int(os.environ.get("K_XHEAD", "0"))       # x tiles quantized before w
TB_POOL = int(os.environ.get("K_TBPOOL", "3"))    # T/B units on Pool out of 5

_CACHE = {}


class _QuantPipe:
    """Software-pipelined BFP quantizer across [128, HF] fp16 half-tile units.

    Stage A (Pool): absmax reduce; (DVE) int16 exponent bit-math -> inv/step.
    Stage T (DVE 3/4, Pool 1/4): y_f16 = x_f16 * inv_bf16 (broadcast).
    Stage R (ACT): y += 1536 (fp16 ulp-1 octave => RNE round to integer).
    Stage S (DVE): m_bf16 = min(y - 1536, 127); m = max(m, -128) (4x ops).
    Stage B (DVE 3/4, Pool 1/4): xq_bf16 = m * step (broadcast).
    Staggered emission (R/S one unit late, B two late) keeps engine queues
    free of head-of-line waits.
    """

    def __init__(self, nc, small, magic_p, magic_n, ypool, mpool):
        self.nc = nc
        self.small = small
        self.ypool = ypool
        self.mpool = mpool
        self.mp = magic_p
        self.mn = magic_n
        self.queue = []
        self.nunit = 0

    def _stage_A(self, u):
        nc, dt = self.nc, mybir.dt
        g = HF // J
        x3 = u["src"].rearrange("p (g j) -> p g j", j=J)
        gmax = self.small.tile([P, g], dt.float16, tag="gmax", name="gmax")
        nc.vector.tensor_reduce(gmax[:], x3, mybir.AxisListType.X,
                                mybir.AluOpType.max, apply_absolute_value=True)
        p2 = self.small.tile([P, g], dt.int16, tag="p2", name="p2")
        nc.vector.tensor_scalar(p2[:], gmax[:].bitcast(dt.int16), MASK16,
                                None, mybir.AluOpType.bitwise_and)
        nc.vector.tensor_scalar(p2[:], p2[:], MINN16, None,
                                mybir.AluOpType.max)
        e7 = self.small.tile([P, g], dt.int16, tag="e7", name="e7")
        nc.vector.tensor_scalar(e7[:], p2[:], 3, None,
                                mybir.AluOpType.logical_shift_right)
        inv_b = self.small.tile([P, g], dt.int16, tag="invb", name="invb")
        nc.vector.tensor_scalar(inv_b[:], e7[:], -1, INVC16,
                                mybir.AluOpType.mult, mybir.AluOpType.add)
        step_b = self.small.tile([P, g], dt.int16, tag="stepb", name="stepb")
        nc.vector.tensor_scalar(step_b[:], e7[:], STEPC16, None,
                                mybir.AluOpType.add)
        u["inv_bf"] = inv_b[:].bitcast(dt.bfloat16)
        u["step_bf"] = step_b[:].bitcast(dt.bfloat16)

    def _stage_T(self, u):
        nc, dt = self.nc, mybir.dt
        g = HF // J
        yt = self.ypool.tile([P, HF], dt.float16, tag="y", name="y")
        y3 = yt[:].rearrange("p (g j) -> p g j", j=J)
        x3 = u["src"].rearrange("p (g j) -> p g j", j=J)
        inv_3 = u["inv_bf"].unsqueeze(2).broadcast_to([P, g, J])
        eng = nc.gpsimd if u["idx"] % 5 < TB_POOL else nc.vector
        eng.tensor_tensor(y3, x3, inv_3, mybir.AluOpType.mult)
        u["y"] = yt

    def _stage_RS(self, u):
        nc, dt = self.nc, mybir.dt
        nc.scalar.activation(u["y"][:], u["y"][:],
                             mybir.ActivationFunctionType.Identity,
                             bias=self.mp[:])
        mt = self.mpool.tile([P, HF], dt.bfloat16, tag="m", name="m")
        nc.scalar.activation(mt[:], u["y"][:],
                             mybir.ActivationFunctionType.Identity,
                             bias=self.mn[:])
        nc.vector.tensor_scalar(mt[:], mt[:], 127.0, -128.0,
                                mybir.AluOpType.min, mybir.AluOpType.max)
        u["m"] = mt

    def _stage_B(self, u):
        nc, dt = self.nc, mybir.dt
        g = HF // J
        dst3 = u["dst"].rearrange("p (g j) -> p g j", j=J)
        step_3 = u["step_bf"].unsqueeze(2).broadcast_to([P, g, J])
        eng = nc.gpsimd if (u["idx"] + 2) % 5 < TB_POOL else nc.vector
        eng.tensor_tensor(dst3, u["m"][:].rearrange("p (g j) -> p g j", j=J),
                          step_3, mybir.AluOpType.mult)
        if u.get("post") is not None:
            u["post"]()

    def push(self, src_ap, dst_ap, post=None):
        u = {"src": src_ap, "dst": dst_ap, "post": post, "idx": self.nunit}
        self.nunit += 1
        self._stage_A(u)
        self._stage_T(u)
        self.queue.append(u)
        if len(self.queue) >= 3:
            v = self.queue[-3]
            if not v.get("cr"):
                self._stage_RS(v)
                v["cr"] = True
        if len(self.queue) >= 5:
            v = self.queue.pop(0)
            self._stage_B(v)

    def flush(self):
        for v in self.queue:
            if not v.get("cr"):
                self._stage_RS(v)
                v["cr"] = True
        for v in self.queue:
            self._stage_B(v)
        self.queue = []


def build(reps=1, ablate=None):
    ablate_set = set((ablate or "").split(","))
    dt = mybir.dt
    nc = bacc.Bacc("TRN2", target_bir_lowering=False, debug=False,
                   num_devices=NCORES)
    x_d = nc.dram_tensor("x_own", [XROWS, IN], dt.float16,
                         kind="ExternalInput").ap()
    w_d = nc.dram_tensor("w_own", [WROWS, IN], dt.float16,
                         kind="ExternalInput").ap()
    b_d = nc.dram_tensor("bias_row", [1, WROWS], dt.float32,
                         kind="ExternalInput").ap()
    out_d = nc.dram_tensor("out", [XROWS, WROWS], dt.float32,
                           kind="ExternalOutput").ap()

    # round-robin DMA issue across the two HWDGE-capable engines (SP, ACT)
    dma_rr = [0]

    def dma_eng():
        dma_rr[0] += 1
        return nc.sync if dma_rr[0] % 2 else nc.scalar

    with tile.TileContext(nc) as tc:
        with tc.tile_pool(name="sb", bufs=1) as sb, \
             tc.tile_pool(name="inp", bufs=4) as inp, \
             tc.tile_pool(name="ypool", bufs=4) as ypool, \
             tc.tile_pool(name="mpool", bufs=4) as mpool, \
             tc.tile_pool(name="xqpool", bufs=4) as xqpool, \
             tc.tile_pool(name="xqtp", bufs=4) as xqtp, \
             tc.tile_pool(name="small", bufs=10) as small, \
             tc.tile_pool(name="otp", bufs=3) as otp, \
             tc.tile_pool(name="dramp", bufs=1, space="DRAM") as dramp, \
             tc.tile_pool(name="psum", bufs=8, space="PSUM") as psump:

            # bias * 2 (reference adds bias twice), as a [1, WROWS] fp32r
            # row: folded into each psum group as a K=1 matmul with ones
            bias_sb = sb.tile([1, WROWS], dt.float32)
            nc.sync.dma_start(bias_sb[:], b_d)
            bias2 = sb.tile([1, WROWS], dt.float32r)
            nc.vector.tensor_scalar(bias2[:], bias_sb[:].bitcast(dt.float32r),
                                    2.0, None, mybir.AluOpType.mult)
            ones_r = sb.tile([1, P], dt.float32r)
            nc.vector.memset(ones_r[:].bitcast(dt.float32), 1.0)
            magic_p = sb.tile([P, 1], dt.float32)
            nc.vector.memset(magic_p[:], MAGIC16)
            magic_n = sb.tile([P, 1], dt.float32)
            nc.vector.memset(magic_n[:], -MAGIC16)

            # wqT[i, kt, o]: contraction = kt*128+i, o = per-core out col
            wqT = sb.tile([P, KT, WROWS], dt.bfloat16)

            if not XBAR_SBUF:
                xq_dram = dramp.tile([XROWS, IN], dt.bfloat16)
                wq_dram = dramp.tile([WROWS, IN], dt.bfloat16)

            pipe = _QuantPipe(nc, small, magic_p, magic_n, ypool, mpool)

            for rep in range(reps):
                xqT_tiles = {}
                xq_pending = {}
                w_done = [0] * (WROWS // OHALF)  # completed w units per slice
                mm_done = set()
                mm_wait = []           # (t, oh) waiting on w half
                evac_q = []            # psum groups awaiting evacuation
                ot_tiles = {}
                ot_done = {}

                def do_evac(ps, t, oh):
                    if t not in ot_tiles:
                        ot_tiles[t] = otp.tile([P, WROWS], dt.float32,
                                               tag="ot", name="ot")
                    ot = ot_tiles[t]
                    nc.scalar.activation(
                        ot[:, oh * OHALF:(oh + 1) * OHALF], ps[:],
                        mybir.ActivationFunctionType.Identity)
                    ot_done[t] = ot_done.get(t, 0) + 1
                    if ot_done[t] == WROWS // OHALF:
                        nc.scalar.dma_start(out_d[t * P:(t + 1) * P, :],
                                            ot[:])
                        del ot_tiles[t]

                def flush_evac():
                    while evac_q:
                        ps, t, oh = evac_q.pop(0)
                        do_evac(ps, t, oh)

                def emit_mm(t, oh):
                    mm_done.add((t, oh))
                    xqT = xqT_tiles[t]
                    ps = psump.tile([P, OHALF], dt.float32, tag="ps",
                                    name="ps")
                    nc.tensor.matmul(
                        ps[:], ones_r[:],
                        bias2[:, oh * OHALF:(oh + 1) * OHALF],
                        start=True, stop=False, skip_group_check=True)
                    for kt in range(KT):
                        nc.tensor.matmul(
                            ps[:],
                            xqT[:, kt, :],
                            wqT[:, kt, oh * OHALF:(oh + 1) * OHALF],
                            start=False, stop=(kt == KT - 1),
                            skip_group_check=True,
                        )
                    evac_q.append((ps, t, oh))
                    while len(evac_q) > 1:
                        ps2, t2, oh2 = evac_q.pop(0)
                        do_evac(ps2, t2, oh2)

                def push_w_half(j, h, wt_full):
                    wt = wt_full[:, h * HF:(h + 1) * HF]
                    wqh = xqpool.tile([P, HF], dt.bfloat16, tag="xqh",
                                      name="wqh")

                    def post(wqh=wqh, j=j, h=h):
                        dst = wqT[:, h * KH:(h + 1) * KH,
                                  j * P:(j + 1) * P]
                        if XBAR_SBUF:
                            nc.scalar.dma_start_transpose(dst, wqh[:])
                        else:
                            nc.scalar.dma_start(
                                wq_dram[j * P:(j + 1) * P,
                                        h * HF:(h + 1) * HF], wqh[:])
                            nc.sync.dma_start_transpose(
                                dst, wq_dram[j * P:(j + 1) * P,
                                             h * HF:(h + 1) * HF])
                        q = j // (OHALF // P)
                        w_done[q] += 1
                        if w_done[q] == (OHALF // P) * NH:
                            for (tt, oo) in [p for p in mm_wait
                                             if p[1] == q]:
                                mm_wait.remove((tt, oo))
                                emit_mm(tt, oo)
                    pipe.push(wt, wqh[:], post)

                def push_x_half(t, h, xt_full):
                    xt = xt_full[:, h * HF:(h + 1) * HF]
                    xqh = xqpool.tile([P, HF], dt.bfloat16, tag="xqh",
                                      name="xqh")
                    if t not in xqT_tiles:
                        xqT_tiles[t] = xqtp.tile([P, KT, P], dt.bfloat16,
                                                 tag="xqT", name="xqT")

                    def post(xqh=xqh, t=t, h=h):
                        dst = xqT_tiles[t][:, h * KH:(h + 1) * KH, :]
                        if XBAR_SBUF:
                            nc.scalar.dma_start_transpose(dst, xqh[:])
                        else:
                            nc.scalar.dma_start(
                                xq_dram[t * P:(t + 1) * P,
                                        h * HF:(h + 1) * HF], xqh[:])
                            nc.sync.dma_start_transpose(
                                dst, xq_dram[t * P:(t + 1) * P,
                                             h * HF:(h + 1) * HF])
                        xq_pending[t] = xq_pending.get(t, 0) + 1
                        if xq_pending[t] == NH and "qonly" not in ablate_set:
                            for oh in range(WROWS // OHALF):
                                if w_done[oh] == (OHALF // P) * NH:
                                    emit_mm(t, oh)
                                else:
                                    mm_wait.append((t, oh))
                    pipe.push(xt, xqh[:], post)

                # x0 first so the PE gets work as soon as w pair 0 lands,
                # then 3 w tiles per x tile (w exhausted by x2; deferred
                # quarters stay within the xqT ring depth). Loads are full
                # row-tiles on the SP queue, untouched by xbar/output
                # traffic (ACT queue).
                def load_full(src_ap):
                    tl = inp.tile([P, IN], dt.float16, tag="in", name="in")
                    nc.sync.dma_start(tl[:], src_ap)
                    return tl

                def push_x_tile(t):
                    xt_full = load_full(x_d[t * P:(t + 1) * P, :])
                    for h in range(NH):
                        push_x_half(t, h, xt_full[:])

                # x0-x2 first: their mm quarters drain in ~10us bursts as w
                # pairs land, keeping the PE warm through the w stream
                for t in range(XHEAD):
                    push_x_tile(t)
                for j in range(WT):
                    wt_full = load_full(w_d[j * P:(j + 1) * P, :])
                    for h in range(NH):
                        push_w_half(j, h, wt_full[:])
                # drain the pipe so every deferred mm group (tiles 0..XHEAD-1
                # waiting on late w slices) is emitted BEFORE the xqT ring
                # recycles those tiles' buffers for the x stream below
                pipe.flush()
                for t in range(XHEAD, XT):
                    push_x_tile(t)
                pipe.flush()
                flush_evac()
                assert not mm_wait and len(mm_done) == XT * (WROWS // OHALF)
    nc.compile()
    return nc


def _get_nc():
    if "nc" not in _CACHE:
        _CACHE["nc"] = build()
    return _CACHE["nc"]


def _in_maps(x, weight, bias):
    in_maps = []
    for c in range(NCORES):
        r, k = c // CGRP, c % CGRP
        in_maps.append({
            "x_own": x[XROWS * r:XROWS * (r + 1)],
            "w_own": weight[WROWS * k:WROWS * (k + 1)],
            "bias_row": np.ascontiguousarray(
                bias[WROWS * k:WROWS * (k + 1)].reshape(1, WROWS)),
        })
    return in_maps


def kernel(x, weight, bias, _trace=False):
    nc = _get_nc()
    x = np.asarray(x, dtype=np.float32).astype(np.float16)
    weight = np.asarray(weight, dtype=np.float32).astype(np.float16)
    bias = np.asarray(bias, dtype=np.float32)

    res = run_bass_kernel_spmd(nc, _in_maps(x, weight, bias),
                               core_ids=list(range(NCORES)), trace=_trace)
    out = np.empty((N, OUT), dtype=np.float32)
    for c in range(NCORES):
        r, k = c // CGRP, c % CGRP
        out[XROWS * r:XROWS * (r + 1), WROWS * k:WROWS * (k + 1)] = \
            res.results[c]["out"]
    if _trace:
        return out, res
    return out


def _pjrt_runner(nc):
    """Return fn() that executes nc's NEFF once across the 8 cores."""
    import jax
    from jax.sharding import Mesh, PartitionSpec, NamedSharding
    from jax.experimental.shard_map import shard_map
    from concourse import bass2jax, mybir as mb

    bass2jax.install_neuronx_cc_hook()
    partition_name = (nc.partition_id_tensor.name
                      if nc.partition_id_tensor else None)
    in_names, out_names, out_avals, zero_outs = [], [], [], []
    for alloc in nc.m.functions[0].allocations:
        if not isinstance(alloc, mb.MemoryLocationSet):
            continue
        name = alloc.memorylocations[0].name
        if alloc.kind == "ExternalInput":
            if name != partition_name:
                in_names.append(name)
        elif alloc.kind == "ExternalOutput":
            out_names.append(name)
            shape = tuple(alloc.tensor_shape)
            dtype = mb.dt.np(alloc.dtype)
            out_avals.append(jax.core.ShapedArray(shape, dtype))
            zero_outs.append(np.zeros(shape, dtype))
    n_params = len(in_names)
    all_names = tuple(in_names + out_names
                      + ([partition_name] if partition_name else []))

    def body(*args):
        extra = ([bass2jax.partition_id_tensor()] if partition_name else [])
        outs = bass2jax._bass_exec_p.bind(
            *args, *extra,
            out_avals=tuple(out_avals),
            in_names=all_names,
            out_names=tuple(out_names),
            lowering_input_output_aliases=(),
            sim_require_finite=True,
            sim_require_nnan=True,
            nc=nc,
        )
        return tuple(outs)

    devices = jax.devices()[:NCORES]
    mesh = Mesh(np.asarray(devices), ("core",))
    specs = (PartitionSpec("core"),) * (n_params + len(out_names))
    fn = jax.jit(shard_map(body, mesh=mesh, in_specs=specs,
                           out_specs=(PartitionSpec("core"),) * len(out_names),
                           check_rep=False), keep_unused=True)
    return fn, in_names, zero_outs


def _concat_inputs(in_names, x, weight, bias):
    maps = _in_maps(x, weight, bias)
    return [np.concatenate([maps[c][n] for c in range(NCORES)], axis=0)
            for n in in_names]


def time_kernel(x, weight, bias, chain=17, reps=10):
    """Per-execution device time via in-NEFF rep chaining:
    (wall(chain-rep NEFF) - wall(1-rep NEFF)) / (chain - 1)."""
    import time
    import jax
    from jax.sharding import Mesh, PartitionSpec, NamedSharding

    x = np.asarray(x, dtype=np.float32).astype(np.float16)
    weight = np.asarray(weight, dtype=np.float32).astype(np.float16)
    bias = np.asarray(bias, dtype=np.float32)

    walls = {}
    for k in (1, chain):
        nc = _get_nc() if k == 1 else build(reps=k)
        fn, in_names, zero_outs = _pjrt_runner(nc)
        maps = _in_maps(x, weight, bias)
        concat_in = [np.concatenate([maps[c][n] for c in range(NCORES)],
                                    axis=0) for n in in_names]
        concat_zeros = [np.zeros((NCORES * z.shape[0], *z.shape[1:]), z.dtype)
                        for z in zero_outs]
        mesh = Mesh(np.asarray(jax.devices()[:NCORES]), ("core",))
        sh = NamedSharding(mesh, PartitionSpec("core"))
        concat_in = [jax.device_put(a, sh) for a in concat_in]
        concat_zeros = [jax.device_put(a, sh) for a in concat_zeros]
        out = fn(*concat_in, *concat_zeros)
        jax.block_until_ready(out)
        ts = []
        for _ in range(reps):
            t0 = time.perf_counter()
            out = fn(*concat_in, *concat_zeros)
            jax.block_until_ready(out)
            ts.append(time.perf_counter() - t0)
        walls[k] = ts
    per_exec = (min(walls[chain]) - min(walls[1])) / (chain - 1)
    return per_exec, walls


# revision 36
# speedup vs baseline: 1.0872x; 1.0872x over previous
"""BFP-quantized linear (nn_BFPLinear) on 8 Trainium2 NeuronCores.

Math (reference): xq = bfp_quant8_g64(x); wq = bfp_quant8_g64(weight);
out = xq @ wq.T + 2*bias.

Sharding (2 row-groups x 4 col-groups grid, core c = 4r+k), NO collectives:
core (r, k) owns x rows [2048r, 2048(r+1)) and w rows [1024k, 1024(k+1)),
quantizes both locally (w redundantly per column pair - cheaper than any
exchange) and writes out[2048r:.., 1024k:..] in natural order; the host
just places the 8 blocks.

Inputs are shipped as fp16 (one extra RNE rounding vs the fp32 reference;
measured output rel-l2 4.2e-3, gate 2e-2). Quantization per 64-group:
  gmax = absmax (DVE reduce, fp16)
  exponent bit-math on int16 views -> inv = 2^(7-e), step = 2^(e-7) in bf16
  y = x * inv (fp16, broadcast multiply, DVE/Pool)
  y += 1536.0 on ACT: fp16 ulp-1 octave [1024, 2048) => RNE round-to-int
  m = (y - 1536) -> bf16 (ACT), clip to [-128, 127] (DVE 4x tensor_scalar)
  xq = m * step (bf16, broadcast multiply, DVE/Pool)
All exact given the fp16 input: products/magic sums stay on representable
grids, m*step is exact in bf16 (|m| <= 128, step a power of 2).

Layout: the contraction dim must sit on partitions for both matmul
operands, so xq/wq tiles [128 rows, 4096] go through the DMA-xbar
transpose SBUF->SBUF (no HBM round trip) into [128 in, kt, row] tiles.

Matmul: out tile [128, 512] accumulates 2*bias via a K=1 fp32r matmul
against a ones-vector, then 32 bf16 matmuls over kt; kt is split in two
passes (kt<16 needs only the first half of each w tile) so the PE starts
after just 8 quantization units. PSUM groups are evacuated on ACT one
group late (no head-of-line blocking) into a per-row-tile staging buffer,
written out with a single DMA.
"""
import sys

sys.path.insert(0, "/opt/trn_rl_repo")

import numpy as np

import concourse.bass as bass
import concourse.tile as tile
from concourse import mybir, bacc
from concourse.bass_utils import run_bass_kernel_spmd

# problem shape (hardcoded; kernel.py must be self-contained)
N = 4096
IN = 4096
OUT = 4096
NCORES = 8
RGRP = 2                 # row groups (x sharded 2-way)
CGRP = 4                 # col groups (w sharded 4-way)
XROWS = N // RGRP        # 2048 x rows per core
WROWS = OUT // CGRP      # 1024 w rows per core
P = 128
J = 64                   # bfp group size
KT = IN // P             # 32 contraction k-tiles
XT = XROWS // P          # 16 x row-tiles per core
WT = WROWS // P          # 8 w row-tiles per core
HF = 2048                # quantize sub-tile width (along `in`)
NH = IN // HF            # 2 halves per row-tile
KH = HF // P             # 16 k-tiles per half

# fp16 bit-field constants (inputs are shipped as fp16; exponent math is
# done on int16 views, producing bf16 inv/step: bits catch 2^(7-e)/2^(e-7))
MASK16 = 0x7C00          # fp16 exponent field
MINN16 = 0x0400          # smallest normal fp16 (e = -14 floor)
INVC16 = 0x4A80          # bf16 bits: (149 - E16) << 7 == 2^(7-e)
STEPC16 = 0x3480         # bf16 bits: (E16 + 105) << 7 == 2^(e-7)
MAGIC16 = 1536.0         # 1.5 * 2^10: fp16 ulp-1 octave [1024, 2048)

# transpose path: True = SBUF->SBUF xbar (no HBM round trip);
# False = stage quantized tiles in DRAM and read back transposed.
XBAR_SBUF = True

OHALF = 512       # output column slice per psum group
TB_POOL = 3       # T/B broadcast units assigned to Pool, out of every 5

_CACHE = {}


class _QuantPipe:
    """Software-pipelined BFP quantizer across [128, HF] fp16 half-tile units.

    Stage A (Pool): absmax reduce; (DVE) int16 exponent bit-math -> inv/step.
    Stage T (DVE 3/4, Pool 1/4): y_f16 = x_f16 * inv_bf16 (broadcast).
    Stage R (ACT): y += 1536 (fp16 ulp-1 octave => RNE round to integer).
    Stage S (DVE): m_bf16 = min(y - 1536, 127); m = max(m, -128) (4x ops).
    Stage B (DVE 3/4, Pool 1/4): xq_bf16 = m * step (broadcast).
    Staggered emission (R/S one unit late, B two late) keeps engine queues
    free of head-of-line waits.
    """

    def __init__(self, nc, small, magic_p, magic_n, ypool, mpool):
        self.nc = nc
        self.small = small
        self.ypool = ypool
        self.mpool = mpool
        self.mp = magic_p
        self.mn = magic_n
        self.queue = []
        self.nunit = 0

    def _stage_A(self, u):
        nc, dt = self.nc, mybir.dt
        g = HF // J
        x3 = u["src"].rearrange("p (g j) -> p g j", j=J)
        gmax = self.small.tile([P, g], dt.float16, tag="gmax", name="gmax")
        nc.vector.tensor_reduce(gmax[:], x3, mybir.AxisListType.X,
                                mybir.AluOpType.max, apply_absolute_value=True)
        p2 = self.small.tile([P, g], dt.int16, tag="p2", name="p2")
        nc.vector.tensor_scalar(p2[:], gmax[:].bitcast(dt.int16), MASK16,
                                None, mybir.AluOpType.bitwise_and)
        nc.vector.tensor_scalar(p2[:], p2[:], MINN16, None,
                                mybir.AluOpType.max)
        e7 = self.small.tile([P, g], dt.int16, tag="e7", name="e7")
        nc.vector.tensor_scalar(e7[:], p2[:], 3, None,
                                mybir.AluOpType.logical_shift_right)
        inv_b = self.small.tile([P, g], dt.int16, tag="invb", name="invb")
        nc.vector.tensor_scalar(inv_b[:], e7[:], -1, INVC16,
                                mybir.AluOpType.mult, mybir.AluOpType.add)
        step_b = self.small.tile([P, g], dt.int16, tag="stepb", name="stepb")
        nc.vector.tensor_scalar(step_b[:], e7[:], STEPC16, None,
                                mybir.AluOpType.add)
        u["inv_bf"] = inv_b[:].bitcast(dt.bfloat16)
        u["step_bf"] = step_b[:].bitcast(dt.bfloat16)

    def _stage_T(self, u):
        nc, dt = self.nc, mybir.dt
        g = HF // J
        yt = self.ypool.tile([P, HF], dt.float16, tag="y", name="y")
        y3 = yt[:].rearrange("p (g j) -> p g j", j=J)
        x3 = u["src"].rearrange("p (g j) -> p g j", j=J)
        inv_3 = u["inv_bf"].unsqueeze(2).broadcast_to([P, g, J])
        eng = nc.gpsimd if u["idx"] % 5 < TB_POOL else nc.vector
        eng.tensor_tensor(y3, x3, inv_3, mybir.AluOpType.mult)
        u["y"] = yt

    def _stage_RS(self, u):
        nc, dt = self.nc, mybir.dt
        nc.scalar.activation(u["y"][:], u["y"][:],
                             mybir.ActivationFunctionType.Identity,
                             bias=self.mp[:])
        mt = self.mpool.tile([P, HF], dt.bfloat16, tag="m", name="m")
        nc.scalar.activation(mt[:], u["y"][:],
                             mybir.ActivationFunctionType.Identity,
                             bias=self.mn[:])
        nc.vector.tensor_scalar(mt[:], mt[:], 127.0, -128.0,
                                mybir.AluOpType.min, mybir.AluOpType.max)
        u["m"] = mt

    def _stage_B(self, u):
        nc, dt = self.nc, mybir.dt
        g = HF // J
        dst3 = u["dst"].rearrange("p (g j) -> p g j", j=J)
        step_3 = u["step_bf"].unsqueeze(2).broadcast_to([P, g, J])
        eng = nc.gpsimd if (u["idx"] + 2) % 5 < TB_POOL else nc.vector
        eng.tensor_tensor(dst3, u["m"][:].rearrange("p (g j) -> p g j", j=J),
                          step_3, mybir.AluOpType.mult)
        if u.get("post") is not None:
            u["post"]()

    def push(self, src_ap, dst_ap, post=None):
        u = {"src": src_ap, "dst": dst_ap, "post": post, "idx": self.nunit}
        self.nunit += 1
        self._stage_A(u)
        self._stage_T(u)
        self.queue.append(u)
        if len(self.queue) >= 3:
            v = self.queue[-3]
            if not v.get("cr"):
                self._stage_RS(v)
                v["cr"] = True
        if len(self.queue) >= 5:
            v = self.queue.pop(0)
            self._stage_B(v)

    def flush(self):
        for v in self.queue:
            if not v.get("cr"):
                self._stage_RS(v)
                v["cr"] = True
        for v in self.queue:
            self._stage_B(v)
        self.queue = []


def build(reps=1, ablate=None):
    ablate_set = set((ablate or "").split(","))
    dt = mybir.dt
    nc = bacc.Bacc("TRN2", target_bir_lowering=False, debug=False,
                   num_devices=NCORES)
    x_d = nc.dram_tensor("x_own", [XROWS, IN], dt.float16,
                         kind="ExternalInput").ap()
    w_d = nc.dram_tensor("w_own", [WROWS, IN], dt.float16,
                         kind="ExternalInput").ap()
    b_d = nc.dram_tensor("bias_row", [1, WROWS], dt.float32,
                         kind="ExternalInput").ap()
    out_d = nc.dram_tensor("out", [XROWS, WROWS], dt.float32,
                           kind="ExternalOutput").ap()

    # round-robin DMA issue across the two HWDGE-capable engines (SP, ACT)
    dma_rr = [0]

    def dma_eng():
        dma_rr[0] += 1
        return nc.sync if dma_rr[0] % 2 else nc.scalar

    with tile.TileContext(nc) as tc:
        with tc.tile_pool(name="sb", bufs=1) as sb, \
             tc.tile_pool(name="inp", bufs=3) as inp, \
             tc.tile_pool(name="ypool", bufs=4) as ypool, \
             tc.tile_pool(name="mpool", bufs=4) as mpool, \
             tc.tile_pool(name="xqpool", bufs=4) as xqpool, \
             tc.tile_pool(name="xqtp", bufs=4) as xqtp, \
             tc.tile_pool(name="small", bufs=10) as small, \
             tc.tile_pool(name="otp", bufs=3) as otp, \
             tc.tile_pool(name="dramp", bufs=1, space="DRAM") as dramp, \
             tc.tile_pool(name="psum", bufs=8, space="PSUM") as psump:

            # bias * 2 (reference adds bias twice), as a [1, WROWS] fp32r
            # row: folded into each psum group as a K=1 matmul with ones
            bias_sb = sb.tile([1, WROWS], dt.float32)
            nc.sync.dma_start(bias_sb[:], b_d)
            bias2 = sb.tile([1, WROWS], dt.float32r)
            nc.vector.tensor_scalar(bias2[:], bias_sb[:].bitcast(dt.float32r),
                                    2.0, None, mybir.AluOpType.mult)
            ones_r = sb.tile([1, P], dt.float32r)
            nc.vector.memset(ones_r[:].bitcast(dt.float32), 1.0)
            magic_p = sb.tile([P, 1], dt.float32)
            nc.vector.memset(magic_p[:], MAGIC16)
            magic_n = sb.tile([P, 1], dt.float32)
            nc.vector.memset(magic_n[:], -MAGIC16)

            # wqT[i, kt, o]: contraction = kt*128+i, o = per-core out col
            wqT = sb.tile([P, KT, WROWS], dt.bfloat16)

            if not XBAR_SBUF:
                xq_dram = dramp.tile([XROWS, IN], dt.bfloat16)
                wq_dram = dramp.tile([WROWS, IN], dt.bfloat16)

            pipe = _QuantPipe(nc, small, magic_p, magic_n, ypool, mpool)

            for rep in range(reps):
                xqT_tiles = {}
                xq_pending = {}
                w_h_done = [0, 0]      # completed w units per kt-half
                mm_state = {}          # t -> 1 after pass1, 2 after pass2
                ps_tiles = {}          # t -> [ps(oh0), ps(oh1)]
                p1_wait = []           # tiles waiting on w h0
                p2_wait = []           # tiles waiting on w h1 / pass1
                evac_q = []            # psum groups awaiting evacuation
                ot_tiles = {}
                ot_done = {}

                def do_evac(ps, t, oh):
                    if t not in ot_tiles:
                        ot_tiles[t] = otp.tile([P, WROWS], dt.float32,
                                               tag="ot", name="ot")
                    ot = ot_tiles[t]
                    nc.scalar.activation(
                        ot[:, oh * OHALF:(oh + 1) * OHALF], ps[:],
                        mybir.ActivationFunctionType.Identity)
                    ot_done[t] = ot_done.get(t, 0) + 1
                    if ot_done[t] == WROWS // OHALF:
                        nc.scalar.dma_start(out_d[t * P:(t + 1) * P, :],
                                            ot[:])
                        del ot_tiles[t]

                def flush_evac():
                    while evac_q:
                        ps, t, oh = evac_q.pop(0)
                        do_evac(ps, t, oh)

                def emit_pass1(t):
                    mm_state[t] = 1
                    xqT = xqT_tiles[t]
                    ps_tiles[t] = []
                    for oh in range(WROWS // OHALF):
                        ps = psump.tile([P, OHALF], dt.float32, tag="ps",
                                        name="ps")
                        ps_tiles[t].append(ps)
                        nc.tensor.matmul(
                            ps[:], ones_r[:],
                            bias2[:, oh * OHALF:(oh + 1) * OHALF],
                            start=True, stop=False, skip_group_check=True)
                        for kt in range(KT // 2):
                            nc.tensor.matmul(
                                ps[:],
                                xqT[:, kt, :],
                                wqT[:, kt, oh * OHALF:(oh + 1) * OHALF],
                                start=False, stop=False,
                                skip_group_check=True,
                            )

                def emit_pass2(t):
                    mm_state[t] = 2
                    xqT = xqT_tiles[t]
                    for oh in range(WROWS // OHALF):
                        ps = ps_tiles[t][oh]
                        for kt in range(KT // 2, KT):
                            nc.tensor.matmul(
                                ps[:],
                                xqT[:, kt, :],
                                wqT[:, kt, oh * OHALF:(oh + 1) * OHALF],
                                start=False, stop=(kt == KT - 1),
                                skip_group_check=True,
                            )
                        evac_q.append((ps, t, oh))
                        while len(evac_q) > 1:
                            ps2, t2, oh2 = evac_q.pop(0)
                            do_evac(ps2, t2, oh2)
                    del ps_tiles[t]

                def push_w_half(j, h):
                    wt_t = inp.tile([P, HF], dt.float16, tag="inw",
                                    name="inw")
                    nc.sync.dma_start(wt_t[:], w_d[j * P:(j + 1) * P,
                                                   h * HF:(h + 1) * HF])
                    wt = wt_t[:]
                    wqh = xqpool.tile([P, HF], dt.bfloat16, tag="xqh",
                                      name="wqh")

                    def post(wqh=wqh, j=j, h=h):
                        dst = wqT[:, h * KH:(h + 1) * KH,
                                  j * P:(j + 1) * P]
                        if XBAR_SBUF:
                            nc.scalar.dma_start_transpose(dst, wqh[:])
                        else:
                            nc.scalar.dma_start(
                                wq_dram[j * P:(j + 1) * P,
                                        h * HF:(h + 1) * HF], wqh[:])
                            nc.sync.dma_start_transpose(
                                dst, wq_dram[j * P:(j + 1) * P,
                                             h * HF:(h + 1) * HF])
                        w_h_done[h] += 1
                        if w_h_done[h] == WT:
                            if h == 0:
                                for tt in list(p1_wait):
                                    p1_wait.remove(tt)
                                    emit_pass1(tt)
                                    if tt in p2_ready:
                                        emit_pass2(tt)
                            else:
                                for tt in list(p2_wait):
                                    p2_wait.remove(tt)
                                    if mm_state.get(tt) == 1:
                                        emit_pass2(tt)
                                    else:
                                        p2_ready.add(tt)
                    pipe.push(wt, wqh[:], post)

                def push_x_half(t, h, xt_full):
                    xt = xt_full[:, h * HF:(h + 1) * HF]
                    xqh = xqpool.tile([P, HF], dt.bfloat16, tag="xqh",
                                      name="xqh")
                    if t not in xqT_tiles:
                        xqT_tiles[t] = xqtp.tile([P, KT, P], dt.bfloat16,
                                                 tag="xqT", name="xqT")

                    def post(xqh=xqh, t=t, h=h):
                        dst = xqT_tiles[t][:, h * KH:(h + 1) * KH, :]
                        if XBAR_SBUF:
                            nc.scalar.dma_start_transpose(dst, xqh[:])
                        else:
                            nc.scalar.dma_start(
                                xq_dram[t * P:(t + 1) * P,
                                        h * HF:(h + 1) * HF], xqh[:])
                            nc.sync.dma_start_transpose(
                                dst, xq_dram[t * P:(t + 1) * P,
                                             h * HF:(h + 1) * HF])
                        if "qonly" in ablate_set:
                            return
                        if h == 0:
                            if w_h_done[0] == WT:
                                emit_pass1(t)
                            else:
                                p1_wait.append(t)
                        else:
                            if mm_state.get(t) == 1 and w_h_done[1] == WT:
                                emit_pass2(t)
                            else:
                                p2_wait.append(t)
                    pipe.push(xt, xqh[:], post)

                # x0 first so the PE gets work as soon as w pair 0 lands,
                # then 3 w tiles per x tile (w exhausted by x2; deferred
                # quarters stay within the xqT ring depth). Loads are full
                # row-tiles on the SP queue, untouched by xbar/output
                # traffic (ACT queue).
                def load_full(src_ap):
                    tl = inp.tile([P, IN], dt.float16, tag="in", name="in")
                    nc.sync.dma_start(tl[:], src_ap)
                    return tl

                def push_x_tile(t):
                    xt_full = load_full(x_d[t * P:(t + 1) * P, :])
                    for h in range(NH):
                        push_x_half(t, h, xt_full[:])

                # x0-x2 first: their mm quarters drain in ~10us bursts as w
                # pairs land, keeping the PE warm through the w stream
                p2_ready = set()
                # w h0 first (8 units) unblocks pass1 for every tile; then
                # x tiles stream while w h1 units interleave; flush after the
                # last w unit so deferred passes emit before ring recycling
                for j in range(WT):
                    push_w_half(j, 0)
                nxt_w = 0
                for t in range(3):
                    push_x_tile(t)
                    for _ in range(3):
                        if nxt_w < WT:
                            push_w_half(nxt_w, 1)
                            nxt_w += 1
                while nxt_w < WT:
                    push_w_half(nxt_w, 1)
                    nxt_w += 1
                pipe.flush()
                for t in range(3, XT):
                    push_x_tile(t)
                pipe.flush()
                flush_evac()
                assert not p1_wait and not p2_wait and not ps_tiles
    nc.compile()
    return nc


def _get_nc():
    if "nc" not in _CACHE:
        _CACHE["nc"] = build()
    return _CACHE["nc"]


def _in_maps(x, weight, bias):
    in_maps = []
    for c in range(NCORES):
        r, k = c // CGRP, c % CGRP
        in_maps.append({
            "x_own": x[XROWS * r:XROWS * (r + 1)],
            "w_own": weight[WROWS * k:WROWS * (k + 1)],
            "bias_row": np.ascontiguousarray(
                bias[WROWS * k:WROWS * (k + 1)].reshape(1, WROWS)),
        })
    return in_maps


def kernel(x, weight, bias, _trace=False):
    nc = _get_nc()
    x = np.asarray(x, dtype=np.float32).astype(np.float16)
    weight = np.asarray(weight, dtype=np.float32).astype(np.float16)
    bias = np.asarray(bias, dtype=np.float32)

    res = run_bass_kernel_spmd(nc, _in_maps(x, weight, bias),
                               core_ids=list(range(NCORES)), trace=_trace)
    out = np.empty((N, OUT), dtype=np.float32)
    for c in range(NCORES):
        r, k = c // CGRP, c % CGRP
        out[XROWS * r:XROWS * (r + 1), WROWS * k:WROWS * (k + 1)] = \
            res.results[c]["out"]
    if _trace:
        return out, res
    return out


def _pjrt_runner(nc):
    """Return fn() that executes nc's NEFF once across the 8 cores."""
    import jax
    from jax.sharding import Mesh, PartitionSpec, NamedSharding
    from jax.experimental.shard_map import shard_map
    from concourse import bass2jax, mybir as mb

    bass2jax.install_neuronx_cc_hook()
    partition_name = (nc.partition_id_tensor.name
                      if nc.partition_id_tensor else None)
    in_names, out_names, out_avals, zero_outs = [], [], [], []
    for alloc in nc.m.functions[0].allocations:
        if not isinstance(alloc, mb.MemoryLocationSet):
            continue
        name = alloc.memorylocations[0].name
        if alloc.kind == "ExternalInput":
            if name != partition_name:
                in_names.append(name)
        elif alloc.kind == "ExternalOutput":
            out_names.append(name)
            shape = tuple(alloc.tensor_shape)
            dtype = mb.dt.np(alloc.dtype)
            out_avals.append(jax.core.ShapedArray(shape, dtype))
            zero_outs.append(np.zeros(shape, dtype))
    n_params = len(in_names)
    all_names = tuple(in_names + out_names
                      + ([partition_name] if partition_name else []))

    def body(*args):
        extra = ([bass2jax.partition_id_tensor()] if partition_name else [])
        outs = bass2jax._bass_exec_p.bind(
            *args, *extra,
            out_avals=tuple(out_avals),
            in_names=all_names,
            out_names=tuple(out_names),
            lowering_input_output_aliases=(),
            sim_require_finite=True,
            sim_require_nnan=True,
            nc=nc,
        )
        return tuple(outs)

    devices = jax.devices()[:NCORES]
    mesh = Mesh(np.asarray(devices), ("core",))
    specs = (PartitionSpec("core"),) * (n_params + len(out_names))
    fn = jax.jit(shard_map(body, mesh=mesh, in_specs=specs,
                           out_specs=(PartitionSpec("core"),) * len(out_names),
                           check_rep=False), keep_unused=True)
    return fn, in_names, zero_outs


def _concat_inputs(in_names, x, weight, bias):
    maps = _in_maps(x, weight, bias)
    return [np.concatenate([maps[c][n] for c in range(NCORES)], axis=0)
            for n in in_names]


def time_kernel(x, weight, bias, chain=17, reps=12):
    """Per-execution device time via in-NEFF rep chaining, interleaved
    median differencing: (wall(chain-rep NEFF) - wall(1-rep NEFF)) /
    (chain - 1). Interleaving + medians defend against tunnel-floor
    drift and outliers."""
    import time
    import statistics
    import jax
    from jax.sharding import Mesh, PartitionSpec, NamedSharding

    x = np.asarray(x, dtype=np.float32).astype(np.float16)
    weight = np.asarray(weight, dtype=np.float32).astype(np.float16)
    bias = np.asarray(bias, dtype=np.float32)

    runners = {}
    for k in (1, chain):
        nc = _get_nc() if k == 1 else build(reps=k)
        fn, in_names, zero_outs = _pjrt_runner(nc)
        maps = _in_maps(x, weight, bias)
        concat_in = [np.concatenate([maps[c][n] for c in range(NCORES)],
                                    axis=0) for n in in_names]
        concat_zeros = [np.zeros((NCORES * z.shape[0], *z.shape[1:]), z.dtype)
                        for z in zero_outs]
        mesh = Mesh(np.asarray(jax.devices()[:NCORES]), ("core",))
        sh = NamedSharding(mesh, PartitionSpec("core"))
        concat_in = [jax.device_put(a, sh) for a in concat_in]
        concat_zeros = [jax.device_put(a, sh) for a in concat_zeros]
        out = fn(*concat_in, *concat_zeros)
        jax.block_until_ready(out)
        runners[k] = (fn, concat_in, concat_zeros)

    walls = {1: [], chain: []}
    for _ in range(reps):
        for k in (1, chain):
            fn, ci, cz = runners[k]
            t0 = time.perf_counter()
            out = fn(*ci, *cz)
            jax.block_until_ready(out)
            walls[k].append(time.perf_counter() - t0)
    per_exec = (statistics.median(walls[chain])
                - statistics.median(walls[1])) / (chain - 1)
    return per_exec, walls
